# revision 2
# baseline (speedup 1.0000x reference)
"""Causal MHA block (QKV proj + SDPA + out proj) on 8 TRN2 cores — v4.

Sharding: batch (4) x head-group (2 groups of 8 heads). Core c handles batch
c//2, heads [g*8, g*8+8) with g = c%2.

Structure (per core):
  - QKV projection for its 8 heads (bf16 matmuls, fp32 PSUM accum). Only the
    pair-0 q/k projection and the first 4 V tiles are emitted up front; the
    REST of the projection matmuls are drip-fed as background filler inside
    the attention loop, so the PE stays busy while the Exp activations (the
    attention-phase bottleneck) run.
  - Causal attention in the S^T orientation (k on partitions), softmax
    denominators via a fused ones-row in the PV matmul. Inner loop pipelined
    per 128-key tile: scores_j+1 is emitted before PV_j so the ACT engine
    never waits on the PE. Diagonal tiles use column-restricted matmuls
    (queries left of the band are skipped).
  - O^T exchange with the pair partner via per-pair bf16 pairwise
    add-ReduceScatters (see below); pair 3 is split by column half with qi
    order [0,2,1,3] so only a ~22us collective remains exposed at the end.
  - Full output projection locally for its own L-half (no output reduce).

The exchange trick: 2-rank concat collectives aren't available, so each core
writes its O^T into BOTH group-slots of a global-dim-ordered buffer, scaled
by a host-supplied 0/1 mask (even core: g0-slot x1, g1-slot x0; odd core
reversed). The pairwise add-ReduceScatter then acts as a concat along the
dim axis while staying SPMD (both cores run the identical program).

Hardcoded shapes per the problem spec: x [4, 2048, 1024], 16 heads, hd 64.
"""
import sys

if '/opt/trn_rl_repo' not in sys.path:
    sys.path.insert(0, '/opt/trn_rl_repo')

import numpy as np
import ml_dtypes

import concourse.bass as bass
import concourse.mybir as mybir
import concourse.tile as tile
from concourse import bacc
from concourse.bass_utils import run_bass_kernel_spmd
from concourse.tile import TileContext

bf16 = ml_dtypes.bfloat16
F32 = mybir.dt.float32
BF16 = mybir.dt.bfloat16

B, L, D, H, HD = 4, 2048, 1024, 16, 64
HPC = 8           # heads per core
GD = HPC * HD     # 512 dims per head-group
LH = L // 2       # 1024, L-half owned by each core after the exchange

RG = [[0, 1], [2, 3], [4, 5], [6, 7]]

_CACHE = {}


def _build_nc():
    nc = bacc.Bacc("TRN2", target_bir_lowering=False, debug=False, num_devices=8)

    xT_d = nc.dram_tensor("xT", [D, L], BF16, kind="ExternalInput").ap()
    wT_d = nc.dram_tensor("wT", [D, 3 * GD], BF16, kind="ExternalInput").ap()
    woT_d = nc.dram_tensor("woT", [D, D], BF16, kind="ExternalInput").ap()
    bqk_d = nc.dram_tensor("bqk", [2 * GD, 1], F32, kind="ExternalInput").ap()
    bv_d = nc.dram_tensor("bv", [128, GD], F32, kind="ExternalInput").ap()
    bob_d = nc.dram_tensor("bob", [128, D], F32, kind="ExternalInput").ap()
    masks_d = nc.dram_tensor("masks", [128, 4 * 512], BF16, kind="ExternalInput").ap()
    m01_d = nc.dram_tensor("m01", [128, 2], F32, kind="ExternalInput").ap()
    y_d = nc.dram_tensor("y", [LH, D], F32, kind="ExternalOutput").ap()

    with TileContext(nc) as tc:
        with (
            tc.tile_pool(name="persist", bufs=1) as persist,
            tc.tile_pool(name="exps", bufs=6) as exps_pool,
            tc.tile_pool(name="small", bufs=2) as small,
            tc.tile_pool(name="ystage", bufs=3) as ystage,
            tc.tile_pool(name="ps_s", bufs=2, space="PSUM") as ps_s,
            tc.tile_pool(name="ps_o", bufs=1, space="PSUM") as ps_o,
            tc.tile_pool(name="ps_op", bufs=2, space="PSUM") as ps_op,
            tc.tile_pool(name="dram", bufs=1, space="DRAM") as dram,
        ):
            # ---- persistent SBUF tensors -------------------------------------
            xT = [persist.tile([128, L], BF16, tag=f"xT{c}", name=f"xT{c}") for c in range(8)]
            wT = [persist.tile([128, 3 * GD], BF16, tag=f"wT{c}", name=f"wT{c}") for c in range(8)]
            qkT = [persist.tile([128, L], BF16, tag=f"qkT{i}", name=f"qkT{i}") for i in range(8)]
            Vt = [persist.tile([128, HPC * (HD + 1)], BF16, tag=f"V{i}", name=f"V{i}")
                  for i in range(16)]
            OTn = [persist.tile([128, L], BF16, tag=f"OTn{p}", name=f"OTn{p}") for p in range(4)]
            woT = [persist.tile([128, D], BF16, tag=f"woT{p}", name=f"woT{p}") for p in range(8)]
            OTm = [persist.tile([128, LH], BF16, tag=f"OTm{p}", name=f"OTm{p}") for p in range(8)]
            bqk = persist.tile([128, 8], F32, tag="bqk")
            m01 = persist.tile([128, 2], F32, tag="m01")
            bv = persist.tile([128, GD], F32, tag="bv")
            bob = persist.tile([128, D], F32, tag="bob")
            masks = persist.tile([128, 4 * 512], BF16, tag="masks")

            # ---- input loads -------------------------------------------------
            # SP queue: what's needed to start compute, coarse-grained after.
            for c in range(8):
                for dt in (0, 4):
                    nc.sync.dma_start(out=wT[c][:, dt * 128:(dt + 1) * 128],
                                      in_=wT_d[c * 128:(c + 1) * 128,
                                               dt * 128:(dt + 1) * 128])
            for c in range(8):
                nc.sync.dma_start(out=xT[c][:, 0:512],
                                  in_=xT_d[c * 128:(c + 1) * 128, 0:512])
            for c in range(8):
                nc.sync.dma_start(out=xT[c][:, 512:2048],
                                  in_=xT_d[c * 128:(c + 1) * 128, 512:2048])
            for c in range(8):
                nc.sync.dma_start(out=wT[c][:, 128:512],
                                  in_=wT_d[c * 128:(c + 1) * 128, 128:512])
                nc.sync.dma_start(out=wT[c][:, 640:1536],
                                  in_=wT_d[c * 128:(c + 1) * 128, 640:1536])
            # Pool queue: biases, masks, out-proj weights (needed late)
            for dt in range(8):
                nc.gpsimd.dma_start(out=bqk[:, dt:dt + 1],
                                    in_=bqk_d[dt * 128:(dt + 1) * 128, :])
            nc.gpsimd.dma_start(out=bv, in_=bv_d[:, :])
            nc.gpsimd.dma_start(out=masks, in_=masks_d[:, :])
            nc.gpsimd.dma_start(out=m01, in_=m01_d[:, :])
            nc.gpsimd.dma_start(out=bob, in_=bob_d[:, :])
            for p in range(8):
                nc.gpsimd.dma_start(out=woT[p], in_=woT_d[p * 128:(p + 1) * 128, :])

            # ---- projection emit helpers -------------------------------------
            def qk_chain_unit(dt, lsb, ci, ps_ref):
                # two accumulation matmuls of the 8-long chain; evac on last
                if ci == 0:
                    ps_ref[0] = ps_op.tile([128, 512], F32, name="ps_proj")
                ps = ps_ref[0]
                for c in (ci, ci + 1):
                    nc.tensor.matmul(
                        ps[:],
                        lhsT=wT[c][:, dt * 128:(dt + 1) * 128],
                        rhs=xT[c][:, lsb * 512:(lsb + 1) * 512],
                        start=(c == 0), stop=(c == 7),
                    )
                if ci == 6:
                    nc.vector.tensor_scalar_add(
                        qkT[dt][:, lsb * 512:(lsb + 1) * 512], ps[:],
                        bqk[:, dt:dt + 1],
                    )

            def v_chain_unit(lb, ci, ps_ref):
                if ci == 0:
                    ps_ref[0] = ps_op.tile([128, 512], F32, name="ps_proj")
                ps = ps_ref[0]
                for c in (ci, ci + 1):
                    nc.tensor.matmul(
                        ps[:],
                        lhsT=xT[c][:, lb * 128:(lb + 1) * 128],
                        rhs=wT[c][:, 1024:1536],
                        start=(c == 0), stop=(c == 7),
                    )
                if ci == 6:
                    v_grp = Vt[lb][:].rearrange("p (h c) -> p h c", c=HD + 1)
                    nc.vector.tensor_add(
                        v_grp[:, :, 0:HD],
                        ps[:].rearrange("p (h c) -> p h c", c=HD),
                        bv[:].rearrange("p (h c) -> p h c", c=HD),
                    )
                    nc.vector.memset(v_grp[:, :, HD:HD + 1], 1.0)

            def qk_proj_units(p):
                units = []
                for dt in (p, 4 + p):
                    for lsb in range(4):
                        ps_ref = [None]
                        for ci in (0, 2, 4, 6):
                            units.append((qk_chain_unit, (dt, lsb, ci, ps_ref)))
                return units

            def v_proj_units(lbs):
                units = []
                for lb in lbs:
                    ps_ref = [None]
                    for ci in (0, 2, 4, 6):
                        units.append((v_chain_unit, (lb, ci, ps_ref)))
                return units

            # foreground: pair-0 q/k projection + first 4 V tiles
            for fn, args in qk_proj_units(0) + v_proj_units(range(4)):
                fn(*args)

            # background filler, drip-fed inside the attention loop.
            # Order matters: a unit must be EMITTED before any attention op
            # that references its output tile (Tile deps follow trace order).
            # With 2 units/j-iter + 4/qi + 8/pair, cumulative emission stays
            # ahead of each consumption point (V4-7 by p0/qi1, ..., qk p1 by
            # p1/qi0, etc).
            bg = (v_proj_units(range(4, 8)) + v_proj_units(range(8, 12))
                  + v_proj_units(range(12, 16)) + qk_proj_units(1)
                  + qk_proj_units(2) + qk_proj_units(3))
            bg_i = [0]

            def bg_pop(n):
                k = 0
                while k < n and bg_i[0] < len(bg):
                    fn, args = bg[bg_i[0]]
                    fn(*args)
                    bg_i[0] += 1
                    k += 1

            # ---- O^T exchange buffers ----------------------------------------
            rs_in = [dram.tile([512, LH], BF16, name=f"rs_in{p}") for p in range(3)]
            rs_out = [dram.tile([256, LH], BF16, name=f"rs_out{p}") for p in range(3)]
            rs_in3 = [dram.tile([512, 512], BF16, name=f"rs_in3{h}") for h in range(2)]
            rs_out3 = [dram.tile([256, 512], BF16, name=f"rs_out3{h}") for h in range(2)]

            def stage_qi(p, qi):
                # write OTn[p][:, qi block] x m01[gi] into the exchange bufs
                for gi in range(2):
                    stg = ystage.tile([128, 512], BF16, tag=f"stg{gi}",
                                      name=f"stg{gi}", bufs=3)
                    nc.vector.tensor_scalar_mul(
                        stg[:], OTn[p][:, qi * 512:(qi + 1) * 512],
                        m01[:, gi:gi + 1])
                    if p < 3:
                        half, co = qi // 2, (qi % 2) * 512
                        nc.sync.dma_start(
                            out=rs_in[p][half * 256 + gi * 128:
                                         half * 256 + (gi + 1) * 128,
                                         co:co + 512],
                            in_=stg[:])
                    else:
                        h, half = qi % 2, qi // 2
                        nc.sync.dma_start(
                            out=rs_in3[h][half * 256 + gi * 128:
                                          half * 256 + (gi + 1) * 128, :],
                            in_=stg[:])

            def exchange(p, h=None):
                i_ap = rs_in[p].opt() if p < 3 else rs_in3[h].opt()
                o_ap = rs_out[p].opt() if p < 3 else rs_out3[h].opt()
                nc.gpsimd.collective_compute(
                    "ReduceScatter", mybir.AluOpType.add,
                    replica_groups=RG, ins=[i_ap], outs=[o_ap],
                )
                for gi in range(2):
                    kp = p + 4 * gi
                    if p < 3:
                        nc.sync.dma_start(
                            out=OTm[kp],
                            in_=rs_out[p][gi * 128:(gi + 1) * 128, :])
                    else:
                        nc.sync.dma_start(
                            out=OTm[kp][:, h * 512:(h + 1) * 512],
                            in_=rs_out3[h][gi * 128:(gi + 1) * 128, :])

            # ---- attention (per-128-key-tile pipeline + bg filler) -----------
            scale = float(1.0 / np.sqrt(HD))

            for p in range(4):
                qi_list = [0, 1, 2, 3] if p < 3 else [0, 2, 1, 3]
                for qn, qi in enumerate(qi_list):
                    nk = 4 * (qi + 1)
                    qsl0 = qi * 512
                    pso = [ps_o.tile([65, 512], F32, tag=f"o{hi}", name=f"pso{hi}")
                           for hi in range(2)]
                    pend = None      # deferred PV of the previous j
                    for j in range(nk):
                        r = j - 4 * qi
                        lo = 128 * r if r > 0 else 0   # causal column cutoff
                        psj = ps_s.tile([128, 1024], F32, name="psj")
                        for hi in range(2):
                            hh = slice(hi * 64, (hi + 1) * 64)
                            nc.tensor.matmul(
                                psj[:, hi * 512 + lo:(hi + 1) * 512],
                                lhsT=qkT[4 + p][hh, j * 128:(j + 1) * 128],
                                rhs=qkT[p][hh, qsl0 + lo:qsl0 + 512],
                                start=True, stop=True,
                                tile_position=(64 * hi, 0),
                            )
                        expt = exps_pool.tile([128, 1024], BF16, tag="exps",
                                              name="expt")
                        if lo == 0:
                            nc.scalar.activation(
                                expt[:], psj[:],
                                mybir.ActivationFunctionType.Exp,
                                scale=scale,
                            )
                        else:
                            for hi in range(2):
                                sl = slice(hi * 512 + lo, (hi + 1) * 512)
                                nc.scalar.activation(
                                    expt[:, sl], psj[:, sl],
                                    mybir.ActivationFunctionType.Exp,
                                    scale=scale,
                                )
                        bg_pop(2)      # fill the PE while ACT_{j-1} drains
                        if pend is not None:
                            pend()     # PV_{j-1}
                            pend = None
                        if r >= 0:      # diagonal k-tile: apply causal mask
                            for hi in range(2):
                                sl = slice(hi * 512 + lo, (hi + 1) * 512)
                                nc.vector.tensor_mul(
                                    expt[:, sl], expt[:, sl],
                                    masks[:, r * 512 + lo:(r + 1) * 512],
                                )

                        def make_pv(j=j, lo=lo, expt=expt, pso=pso, nk=nk, p=p):
                            def pv():
                                for hi in range(2):
                                    hl = 2 * p + hi
                                    nc.tensor.matmul(
                                        pso[hi][:, lo:512],
                                        lhsT=Vt[j][:, hl * 65:hl * 65 + 65],
                                        rhs=expt[:, hi * 512 + lo:(hi + 1) * 512],
                                        start=(j == 0), stop=(j == nk - 1),
                                    )
                            return pv
                        pend = make_pv()
                    pend()
                    # normalize: O^T[hd, q] / rowsum (ones row of pso)
                    for hi in range(2):
                        rec = small.tile([1, 512], F32, tag="rec", name="rec")
                        nc.vector.reciprocal(rec[:], pso[hi][64:65, :])
                        bc = small.tile([64, 512], F32, tag="bc", name="bc")
                        nc.gpsimd.partition_broadcast(bc[:], rec[:], channels=64)
                        if hi == 0:
                            nc.vector.tensor_mul(
                                OTn[p][0:64, qsl0:qsl0 + 512],
                                pso[hi][0:64, :], bc[:])
                        else:
                            tmp = small.tile([64, 512], BF16, tag="tmp", name="tmp")
                            nc.vector.tensor_mul(tmp[:], pso[hi][0:64, :], bc[:])
                            nc.sync.dma_start(
                                out=OTn[p][64:128, qsl0:qsl0 + 512], in_=tmp[:])
                    stage_qi(p, qi)
                    bg_pop(4)
                    if p == 3 and qn == 1:
                        exchange(3, h=0)     # {qi0, qi2}: overlaps qi1+qi3
                if p < 3:
                    exchange(p)
                    bg_pop(8)
            exchange(3, h=1)                 # {qi1, qi3}: the only exposed one
            bg_pop(len(bg))

            # ---- full out-projection on own L-half ---------------------------
            # lb 0-3 need only the h=0 half of pair 3 -> can run during E3b.
            kp_order = [0, 4, 1, 5, 2, 6, 3, 7]
            for lb in range(8):
                for nh in range(2):
                    ps = ps_op.tile([128, 512], F32, name="ps_proj")
                    for i, kp in enumerate(kp_order):
                        nc.tensor.matmul(
                            ps[:],
                            lhsT=OTm[kp][:, lb * 128:(lb + 1) * 128],
                            rhs=woT[kp][:, nh * 512:(nh + 1) * 512],
                            start=(i == 0), stop=(i == 7),
                        )
                    yb = ystage.tile([128, 512], F32, tag="yb", name="yb")
                    nc.vector.tensor_add(yb[:], ps[:],
                                         bob[:, nh * 512:(nh + 1) * 512])
                    nc.scalar.dma_start(
                        out=y_d[lb * 128:(lb + 1) * 128,
                                nh * 512:(nh + 1) * 512],
                        in_=yb[:],
                    )

    nc.compile()
    return nc


def _prep_core_inputs(c, x, Wqkv, bqkv, Wo, bo, masks_np):
    b, g = c // 2, c % 2
    qs = slice(g * GD, (g + 1) * GD)
    ks = slice(D + g * GD, D + (g + 1) * GD)
    vs = slice(2 * D + g * GD, 2 * D + (g + 1) * GD)
    Wc = np.concatenate([Wqkv[qs], Wqkv[ks], Wqkv[vs]], axis=0)
    return {
        "xT": np.ascontiguousarray(x[b].T).astype(bf16),
        "wT": np.ascontiguousarray(Wc.T).astype(bf16),
        "woT": np.ascontiguousarray(Wo.T).astype(bf16),
        "bqk": np.concatenate([bqkv[qs], bqkv[ks]]).astype(np.float32).reshape(2 * GD, 1),
        "bv": np.tile(bqkv[vs].astype(np.float32), (128, 1)),
        "bob": np.tile(bo.astype(np.float32), (128, 1)),
        "masks": masks_np,
        "m01": np.tile(np.array([1 - g, g], dtype=np.float32), (128, 1)),
    }


def _masks_np():
    m = np.zeros((128, 4 * 512), dtype=bf16)
    kk = np.arange(128)[:, None]
    qq = np.arange(512)[None, :]
    for r in range(4):
        m[:, r * 512:(r + 1) * 512] = (qq >= kk + 128 * r).astype(bf16)
    return m


def _run(inputs, trace=False):
    if "nc" not in _CACHE:
        _CACHE["nc"] = _build_nc()
    nc = _CACHE["nc"]
    x = np.asarray(inputs["x"], dtype=np.float32)
    Wqkv = np.asarray(inputs["Wqkv"], dtype=np.float32)
    bqkv = np.asarray(inputs["bqkv"], dtype=np.float32)
    Wo = np.asarray(inputs["Wo"], dtype=np.float32)
    bo = np.asarray(inputs["bo"], dtype=np.float32)
    masks_np = _masks_np()
    in_maps = [_prep_core_inputs(c, x, Wqkv, bqkv, Wo, bo, masks_np)
               for c in range(8)]
    res = run_bass_kernel_spmd(nc, in_maps, core_ids=list(range(8)), trace=trace)
    out = np.empty((B, L, D), dtype=np.float32)
    for b in range(B):
        out[b, :LH] = res.results[2 * b]["y"]
        out[b, LH:] = res.results[2 * b + 1]["y"]
    return out, res


def kernel(x, mask, Wqkv, bqkv, Wo, bo):
    out, _ = _run({"x": x, "mask": mask, "Wqkv": Wqkv, "bqkv": bqkv,
                   "Wo": Wo, "bo": bo})
    return out


def kernel_traced(x, mask, Wqkv, bqkv, Wo, bo):
    return _run({"x": x, "mask": mask, "Wqkv": Wqkv, "bqkv": bqkv,
                 "Wo": Wo, "bo": bo}, trace=True)


# revision 4
# speedup vs baseline: 1.1452x; 1.1452x over previous
"""Causal MHA block (QKV proj + SDPA + out proj) on 8 TRN2 cores — v13.

Sharding: batch (4) x head-group (2 groups of 8 heads). Core c handles batch
c//2, heads [g*8, g*8+8) with g = c%2.

Structure (per core):
  - QKV projection for its 8 heads (bf16 matmuls, fp32 PSUM accum). Only the
    pair-0 q/k projection and the first 4 V tiles are emitted up front; the
    REST of the projection matmuls are drip-fed as background filler inside
    the attention loop, so the PE stays busy while the Exp activations (the
    attention-phase bottleneck) run.
  - Causal attention in the S^T orientation (k on partitions), softmax
    denominators via a fused ones-row in the PV matmul. Inner loop pipelined
    per 128-key tile: scores_j+1 is emitted before PV_j so the ACT engine
    never waits on the PE. Diagonal tiles use column-restricted matmuls
    (queries left of the band are skipped).
  - O^T exchange with the pair partner via per-pair bf16 pairwise
    add-ReduceScatters (see below); pair 3 is split by column half with qi
    order [0,2,1,3] so only a ~22us collective remains exposed at the end.
  - Full output projection locally for its own L-half (no output reduce).

The exchange trick: 2-rank concat collectives aren't available, so each core
writes its O^T into BOTH group-slots of a global-dim-ordered buffer, scaled
by a host-supplied 0/1 mask (even core: g0-slot x1, g1-slot x0; odd core
reversed). The pairwise add-ReduceScatter then acts as a concat along the
dim axis while staying SPMD (both cores run the identical program).

Hardcoded shapes per the problem spec: x [4, 2048, 1024], 16 heads, hd 64.
"""
import sys

if '/opt/trn_rl_repo' not in sys.path:
    sys.path.insert(0, '/opt/trn_rl_repo')

import numpy as np
import ml_dtypes

import concourse.bass as bass
import concourse.mybir as mybir
import concourse.tile as tile
from concourse.tile_rust import add_dep_helper
from concourse import bacc
from concourse.bass_utils import run_bass_kernel_spmd
from concourse.tile import TileContext

bf16 = ml_dtypes.bfloat16
F32 = mybir.dt.float32
BF16 = mybir.dt.bfloat16

B, L, D, H, HD = 4, 2048, 1024, 16, 64
HPC = 8           # heads per core
GD = HPC * HD     # 512 dims per head-group
LH = L // 2       # 1024, L-half owned by each core after the exchange

RG = [[0, 1], [2, 3], [4, 5], [6, 7]]

_CACHE = {}


def _build_nc():
    nc = bacc.Bacc("TRN2", target_bir_lowering=False, debug=False, num_devices=8)

    xT_d = nc.dram_tensor("xT", [D, L], BF16, kind="ExternalInput").ap()
    wT_d = nc.dram_tensor("wT", [D, 3 * GD], BF16, kind="ExternalInput").ap()
    woT_d = nc.dram_tensor("woT", [D, D], BF16, kind="ExternalInput").ap()
    bqk_d = nc.dram_tensor("bqk", [2 * GD, 1], F32, kind="ExternalInput").ap()
    bv_d = nc.dram_tensor("bv", [128, GD], F32, kind="ExternalInput").ap()
    bob_d = nc.dram_tensor("bob", [128, D], F32, kind="ExternalInput").ap()
    masks_d = nc.dram_tensor("masks", [128, 4 * 512], BF16, kind="ExternalInput").ap()
    m01_d = nc.dram_tensor("m01", [128, 2], F32, kind="ExternalInput").ap()
    y_d = nc.dram_tensor("y", [LH, D], F32, kind="ExternalOutput").ap()

    with TileContext(nc) as tc:
        with (
            tc.tile_pool(name="persist", bufs=1) as persist,
            tc.tile_pool(name="exps", bufs=6) as exps_pool,
            tc.tile_pool(name="small", bufs=2) as small,
            tc.tile_pool(name="ystage", bufs=3) as ystage,
            tc.tile_pool(name="ps_s", bufs=2, space="PSUM") as ps_s,
            tc.tile_pool(name="ps_o", bufs=1, space="PSUM") as ps_o,
            tc.tile_pool(name="ps_op", bufs=2, space="PSUM") as ps_op,
            tc.tile_pool(name="dram", bufs=1, space="DRAM") as dram,
        ):
            # ---- persistent SBUF tensors -------------------------------------
            xT = [persist.tile([128, L], BF16, tag=f"xT{c}", name=f"xT{c}") for c in range(8)]
            wT = [persist.tile([128, 3 * GD], BF16, tag=f"wT{c}", name=f"wT{c}") for c in range(8)]
            qkT = [persist.tile([128, L], BF16, tag=f"qkT{i}", name=f"qkT{i}") for i in range(8)]
            Vt = [persist.tile([128, HPC * (HD + 1)], BF16, tag=f"V{i}", name=f"V{i}")
                  for i in range(16)]
            OTn = [persist.tile([128, L], BF16, tag=f"OTn{p}", name=f"OTn{p}") for p in range(4)]
            woT = [persist.tile([128, D], BF16, tag=f"woT{p}", name=f"woT{p}") for p in range(8)]
            OTm = [persist.tile([128, LH], BF16, tag=f"OTm{p}", name=f"OTm{p}") for p in range(8)]
            bqk = persist.tile([128, 8], F32, tag="bqk")
            m01 = persist.tile([128, 2], F32, tag="m01")
            bv = persist.tile([128, GD], F32, tag="bv")
            bob = persist.tile([128, D], F32, tag="bob")
            masks = persist.tile([128, 4 * 512], BF16, tag="masks")

            # ---- input loads -------------------------------------------------
            # SP queue: what's needed to start compute, coarse-grained after.
            for c in range(8):
                for dt in (0, 4):
                    nc.sync.dma_start(out=wT[c][:, dt * 128:(dt + 1) * 128],
                                      in_=wT_d[c * 128:(c + 1) * 128,
                                               dt * 128:(dt + 1) * 128])
            for c in range(8):
                nc.sync.dma_start(out=xT[c][:, 0:512],
                                  in_=xT_d[c * 128:(c + 1) * 128, 0:512])
            for c in range(8):
                nc.sync.dma_start(out=xT[c][:, 512:2048],
                                  in_=xT_d[c * 128:(c + 1) * 128, 512:2048])
            for c in range(8):
                nc.sync.dma_start(out=wT[c][:, 128:512],
                                  in_=wT_d[c * 128:(c + 1) * 128, 128:512])
                nc.sync.dma_start(out=wT[c][:, 640:1536],
                                  in_=wT_d[c * 128:(c + 1) * 128, 640:1536])
            # Pool queue: biases, masks, out-proj weights (needed late)
            for dt in range(8):
                nc.gpsimd.dma_start(out=bqk[:, dt:dt + 1],
                                    in_=bqk_d[dt * 128:(dt + 1) * 128, :])
            nc.gpsimd.dma_start(out=bv, in_=bv_d[:, :])
            nc.gpsimd.dma_start(out=masks, in_=masks_d[:, :])
            nc.gpsimd.dma_start(out=m01, in_=m01_d[:, :])
            nc.gpsimd.dma_start(out=bob, in_=bob_d[:, :])
            for p in range(8):
                nc.gpsimd.dma_start(out=woT[p], in_=woT_d[p * 128:(p + 1) * 128, :])

            # ---- projection emit helpers -------------------------------------
            def qk_chain_unit(dt, lsb, ci, ps_ref):
                # two accumulation matmuls of the 8-long chain; evac on last
                if ci == 0:
                    ps_ref[0] = ps_op.tile([128, 512], F32, name="ps_proj")
                ps = ps_ref[0]
                for c in (ci, ci + 1):
                    nc.tensor.matmul(
                        ps[:],
                        lhsT=wT[c][:, dt * 128:(dt + 1) * 128],
                        rhs=xT[c][:, lsb * 512:(lsb + 1) * 512],
                        start=(c == 0), stop=(c == 7),
                    )
                if ci == 6:
                    nc.vector.tensor_scalar_add(
                        qkT[dt][:, lsb * 512:(lsb + 1) * 512], ps[:],
                        bqk[:, dt:dt + 1],
                    )

            def v_chain_unit(lb, ci, ps_ref):
                if ci == 0:
                    ps_ref[0] = ps_op.tile([128, 512], F32, name="ps_proj")
                ps = ps_ref[0]
                for c in (ci, ci + 1):
                    nc.tensor.matmul(
                        ps[:],
                        lhsT=xT[c][:, lb * 128:(lb + 1) * 128],
                        rhs=wT[c][:, 1024:1536],
                        start=(c == 0), stop=(c == 7),
                    )
                if ci == 6:
                    v_grp = Vt[lb][:].rearrange("p (h c) -> p h c", c=HD + 1)
                    nc.vector.tensor_add(
                        v_grp[:, :, 0:HD],
                        ps[:].rearrange("p (h c) -> p h c", c=HD),
                        bv[:].rearrange("p (h c) -> p h c", c=HD),
                    )
                    nc.vector.memset(v_grp[:, :, HD:HD + 1], 1.0)

            def qk_proj_units(p):
                units = []
                for dt in (p, 4 + p):
                    for lsb in range(4):
                        ps_ref = [None]
                        for ci in (0, 2, 4, 6):
                            units.append((qk_chain_unit, (dt, lsb, ci, ps_ref)))
                return units

            def v_proj_units(lbs):
                units = []
                for lb in lbs:
                    ps_ref = [None]
                    for ci in (0, 2, 4, 6):
                        units.append((v_chain_unit, (lb, ci, ps_ref)))
                return units

            # foreground: pair-0 q/k projection + first 4 V tiles
            for fn, args in qk_proj_units(0) + v_proj_units(range(4)):
                fn(*args)

            # background filler, drip-fed inside the attention loop.
            # Order matters: a unit must be EMITTED before any attention op
            # that references its output tile (Tile deps follow trace order).
            # With 2 units/j-iter + 4/qi + 8/pair, cumulative emission stays
            # ahead of each consumption point (V4-7 by p0/qi1, ..., qk p1 by
            # p1/qi0, etc).
            bg = (v_proj_units(range(4, 8)) + v_proj_units(range(8, 12))
                  + v_proj_units(range(12, 16)) + qk_proj_units(1)
                  + qk_proj_units(2) + qk_proj_units(3))
            bg_i = [0]

            def bg_pop(n):
                k = 0
                while k < n and bg_i[0] < len(bg):
                    fn, args = bg[bg_i[0]]
                    fn(*args)
                    bg_i[0] += 1
                    k += 1

            # ---- O^T exchange buffers ----------------------------------------
            rs_in = [dram.tile([512, LH], BF16, name=f"rs_in{p}") for p in range(3)]
            rs_out = [dram.tile([256, LH], BF16, name=f"rs_out{p}") for p in range(3)]
            rs_in3 = [dram.tile([512, 512], BF16, name=f"rs_in3{h}") for h in range(2)]
            rs_out3 = [dram.tile([256, 512], BF16, name=f"rs_out3{h}") for h in range(2)]

            def stage_qi(p, qi):
                # write OTn[p][:, qi block] x m01[gi] into the exchange bufs
                for gi in range(2):
                    stg = ystage.tile([128, 512], BF16, tag=f"stg{gi}",
                                      name=f"stg{gi}", bufs=3)
                    nc.vector.tensor_scalar_mul(
                        stg[:], OTn[p][:, qi * 512:(qi + 1) * 512],
                        m01[:, gi:gi + 1])
                    if p < 3:
                        half, co = qi // 2, (qi % 2) * 512
                        nc.sync.dma_start(
                            out=rs_in[p][half * 256 + gi * 128:
                                         half * 256 + (gi + 1) * 128,
                                         co:co + 512],
                            in_=stg[:])
                    else:
                        h, half = qi % 2, qi // 2
                        nc.sync.dma_start(
                            out=rs_in3[h][half * 256 + gi * 128:
                                          half * 256 + (gi + 1) * 128, :],
                            in_=stg[:])

            def exchange(p, h=None):
                i_ap = rs_in[p].opt() if p < 3 else rs_in3[h].opt()
                o_ap = rs_out[p].opt() if p < 3 else rs_out3[h].opt()
                nc.gpsimd.collective_compute(
                    "ReduceScatter", mybir.AluOpType.add,
                    replica_groups=RG, ins=[i_ap], outs=[o_ap],
                )

            def load_otm(p, h=None):
                # deferred OTm readback (a DMA that waits on a collective
                # would block the whole in-order SP queue if issued inline)
                # Pool queue: mid-attention it holds only collectives, so an
                # OTm DMA parked on E_p can't stall the SP/DVE pipelines (the
                # scheduler hoists these ahead of later SP DMAs otherwise).
                for gi in range(2):
                    kp = p + 4 * gi
                    if p < 3:
                        nc.gpsimd.dma_start(
                            out=OTm[kp],
                            in_=rs_out[p][gi * 128:(gi + 1) * 128, :])
                    else:
                        nc.gpsimd.dma_start(
                            out=OTm[kp][:, h * 512:(h + 1) * 512],
                            in_=rs_out3[h][gi * 128:(gi + 1) * 128, :])

            # ---- attention (per-128-key-tile pipeline + bg filler) -----------
            scale = float(1.0 / np.sqrt(HD))
            gate = [None]   # last PV matmul; gates out-proj chain starts

            for p in range(4):
                qi_list = [0, 1, 2, 3] if p < 3 else [0, 2, 1, 3]
                for qn, qi in enumerate(qi_list):
                    nk = 4 * (qi + 1)
                    qsl0 = qi * 512
                    pso = [ps_o.tile([65, 512], F32, tag=f"o{hi}", name=f"pso{hi}")
                           for hi in range(2)]
                    pend = None      # deferred PV of the previous j
                    for j in range(nk):
                        r = j - 4 * qi
                        lo = 128 * r if r > 0 else 0   # causal column cutoff
                        psj = ps_s.tile([128, 1024], F32, name="psj")
                        for hi in range(2):
                            hh = slice(hi * 64, (hi + 1) * 64)
                            nc.tensor.matmul(
                                psj[:, hi * 512 + lo:(hi + 1) * 512],
                                lhsT=qkT[4 + p][hh, j * 128:(j + 1) * 128],
                                rhs=qkT[p][hh, qsl0 + lo:qsl0 + 512],
                                start=True, stop=True,
                                tile_position=(64 * hi, 0),
                            )
                        expt = exps_pool.tile([128, 1024], BF16, tag="exps",
                                              name="expt")
                        if lo == 0:
                            nc.scalar.activation(
                                expt[:], psj[:],
                                mybir.ActivationFunctionType.Exp,
                                scale=scale,
                            )
                        else:
                            for hi in range(2):
                                sl = slice(hi * 512 + lo, (hi + 1) * 512)
                                nc.scalar.activation(
                                    expt[:, sl], psj[:, sl],
                                    mybir.ActivationFunctionType.Exp,
                                    scale=scale,
                                )
                        bg_pop(2)      # fill the PE while ACT_{j-1} drains
                        if pend is not None:
                            pend()     # PV_{j-1}
                            pend = None
                        if r >= 0:      # diagonal k-tile: apply causal mask
                            for hi in range(2):
                                sl = slice(hi * 512 + lo, (hi + 1) * 512)
                                nc.vector.tensor_mul(
                                    expt[:, sl], expt[:, sl],
                                    masks[:, r * 512 + lo:(r + 1) * 512],
                                )

                        def make_pv(j=j, lo=lo, expt=expt, pso=pso, nk=nk, p=p):
                            def pv():
                                for hi in range(2):
                                    hl = 2 * p + hi
                                    gate[0] = nc.tensor.matmul(
                                        pso[hi][:, lo:512],
                                        lhsT=Vt[j][:, hl * 65:hl * 65 + 65],
                                        rhs=expt[:, hi * 512 + lo:(hi + 1) * 512],
                                        start=(j == 0), stop=(j == nk - 1),
                                    )
                            return pv
                        pend = make_pv()
                    pend()
                    # evacuate pso to SBUF immediately (DVE only) so the PSUM
                    # slot frees even when the Pool queue is parked on a
                    # 28us collective; normalize works on the copy.
                    cps = [None, None]
                    for hi in range(2):
                        cps[hi] = small.tile([65, 512], F32, tag=f"cp{hi}",
                                             name=f"cp{hi}", bufs=2)
                        nc.vector.tensor_copy(cps[hi][:], pso[hi][:])
                    # normalize: O^T[hd, q] / rowsum (ones row of pso)
                    for hi in range(2):
                        rec = small.tile([1, 512], F32, tag="rec", name="rec")
                        nc.vector.reciprocal(rec[:], cps[hi][64:65, :])
                        bc = small.tile([64, 512], F32, tag="bc", name="bc")
                        # partition-broadcast via a DRAM bounce + stride-0 DMA
                        # on the SP queue: gpsimd's InstPartitionBroadcast
                        # would queue on Pool behind a 28us collective and
                        # stall the DVE stream.
                        rd = dram.tile([1, 512], F32, name="rec_d", tag="rec_d",
                                       bufs=4)
                        nc.sync.dma_start(out=rd[:], in_=rec[:])
                        rec_b = bass.AP(rd[:].tensor, rd[:].offset,
                                        [[0, 64], [1, 512]])
                        nc.sync.dma_start(out=bc[:], in_=rec_b)
                        if hi == 0:
                            nc.vector.tensor_mul(
                                OTn[p][0:64, qsl0:qsl0 + 512],
                                cps[hi][0:64, :], bc[:])
                        else:
                            tmp = small.tile([64, 512], BF16, tag="tmp", name="tmp")
                            nc.vector.tensor_mul(tmp[:], cps[hi][0:64, :], bc[:])
                            nc.sync.dma_start(
                                out=OTn[p][64:128, qsl0:qsl0 + 512], in_=tmp[:])
                    stage_qi(p, qi)
                    bg_pop(4)
                    if p == 3 and qn == 1:
                        exchange(3, h=0)     # {qi0, qi2}: overlaps qi1+qi3
                        load_otm(3, h=0)
                if p < 3:
                    exchange(p)
                    load_otm(p)
                    bg_pop(8)
            exchange(3, h=1)                 # {qi1, qi3}: the only exposed one
            bg_pop(len(bg))

            # ---- full out-projection on own L-half ---------------------------
            # lb 0-3 need only the h=0 half of pair 3 -> can run during E3b.
            kp_order = [0, 4, 1, 5, 2, 6, 3, 7]
            for lb in range(8):
                if lb == 0:
                    load_otm(3, h=1)
                for nh in range(2):
                    ps = ps_op.tile([128, 512], F32, name="ps_proj")
                    for i, kp in enumerate(kp_order):
                        mi = nc.tensor.matmul(
                            ps[:],
                            lhsT=OTm[kp][:, lb * 128:(lb + 1) * 128],
                            rhs=woT[kp][:, nh * 512:(nh + 1) * 512],
                            start=(i == 0), stop=(i == 7),
                        )
                        if i == 0 and gate[0] is not None:
                            add_dep_helper(
                                mi.ins, gate[0].ins, sync=False,
                                reason="keep out-proj out of attention stream")
                    yb = ystage.tile([128, 512], F32, tag="yb", name="yb")
                    nc.vector.tensor_add(yb[:], ps[:],
                                         bob[:, nh * 512:(nh + 1) * 512])
                    nc.scalar.dma_start(
                        out=y_d[lb * 128:(lb + 1) * 128,
                                nh * 512:(nh + 1) * 512],
                        in_=yb[:],
                    )

    nc.compile()
    return nc


def _prep_core_inputs(c, x, Wqkv, bqkv, Wo, bo, masks_np):
    b, g = c // 2, c % 2
    qs = slice(g * GD, (g + 1) * GD)
    ks = slice(D + g * GD, D + (g + 1) * GD)
    vs = slice(2 * D + g * GD, 2 * D + (g + 1) * GD)
    Wc = np.concatenate([Wqkv[qs], Wqkv[ks], Wqkv[vs]], axis=0)
    return {
        "xT": np.ascontiguousarray(x[b].T).astype(bf16),
        "wT": np.ascontiguousarray(Wc.T).astype(bf16),
        "woT": np.ascontiguousarray(Wo.T).astype(bf16),
        "bqk": np.concatenate([bqkv[qs], bqkv[ks]]).astype(np.float32).reshape(2 * GD, 1),
        "bv": np.tile(bqkv[vs].astype(np.float32), (128, 1)),
        "bob": np.tile(bo.astype(np.float32), (128, 1)),
        "masks": masks_np,
        "m01": np.tile(np.array([1 - g, g], dtype=np.float32), (128, 1)),
    }


def _masks_np():
    m = np.zeros((128, 4 * 512), dtype=bf16)
    kk = np.arange(128)[:, None]
    qq = np.arange(512)[None, :]
    for r in range(4):
        m[:, r * 512:(r + 1) * 512] = (qq >= kk + 128 * r).astype(bf16)
    return m


def _run(inputs, trace=False):
    if "nc" not in _CACHE:
        _CACHE["nc"] = _build_nc()
    nc = _CACHE["nc"]
    x = np.asarray(inputs["x"], dtype=np.float32)
    Wqkv = np.asarray(inputs["Wqkv"], dtype=np.float32)
    bqkv = np.asarray(inputs["bqkv"], dtype=np.float32)
    Wo = np.asarray(inputs["Wo"], dtype=np.float32)
    bo = np.asarray(inputs["bo"], dtype=np.float32)
    masks_np = _masks_np()
    in_maps = [_prep_core_inputs(c, x, Wqkv, bqkv, Wo, bo, masks_np)
               for c in range(8)]
    res = run_bass_kernel_spmd(nc, in_maps, core_ids=list(range(8)), trace=trace)
    out = np.empty((B, L, D), dtype=np.float32)
    for b in range(B):
        out[b, :LH] = res.results[2 * b]["y"]
        out[b, LH:] = res.results[2 * b + 1]["y"]
    return out, res


def kernel(x, mask, Wqkv, bqkv, Wo, bo):
    out, _ = _run({"x": x, "mask": mask, "Wqkv": Wqkv, "bqkv": bqkv,
                   "Wo": Wo, "bo": bo})
    return out


def kernel_traced(x, mask, Wqkv, bqkv, Wo, bo):
    return _run({"x": x, "mask": mask, "Wqkv": Wqkv, "bqkv": bqkv,
                 "Wo": Wo, "bo": bo}, trace=True)
